# revision 3
# baseline (speedup 1.0000x reference)
"""Trainium2 Bass kernel for nn_CombinatorialClassifier, v3.

Computation (reference):
    logits = einsum('bf,pqf->bpq', x, W) + b        # [B,P,Q]
    logp   = log_softmax(logits, axis=2)            # [B,P,Q]
    out    = take_along_axis(logp, part_idx, 2)     # [B,P,C]

Shapes: B=256, P=64, Q=128, C=1000, F=2048.  Expert-parallel over P
(8 partitionings per core), full x on every core, no collectives.
~45us HW (from the 45-51us v1), tensor/DVE/ACT all ~90% busy in
their phases.

Design:
  - mains in orientation B (x stationary, fp8 DoubleRow, W scaled by
    32 on host): psum_lin[b,(p,q)] += x_k.T @ W_k, k-major behind the
    input DMA stream.  Bias rides in as PE "opener" matmuls on a
    host-packed ones/bias row.
  - PE clock pre-ramp: fp32 garbage matmuls on a raw uninitialized
    SBUF tensor (zero dependencies) start the DVFS ramp the moment the
    engine is free, so the mains run at 2.4GHz instead of 1.2GHz.
  - gathers are transpose-mode matmuls (lhsT=logpT bf16, moving=
    one-hot bf16, 512+488 cols) writing bf16 PSUM: one 2KB bank per
    (blk,p) and HALF the drain bytes vs f32 psum.  One-hot columns are
    exact in bf16, so the gather is a pure permutation -- no extra
    error.
  - inputs on the sync HWDGE ring in priority order: bias row, 8 xw
    k-tiles, identity, 8 one-hot chunks (bf16 straight from HBM; they
    fill the DMA window between the input and output streams).
  - softmax work is spread: linsc (psum->SBUF descale) + sumexp
    reduces on DVE, exps/ln/neg on ACT; pst->logpT copies batched
    [128,4,128] on DVE.  Emission follows dataflow (tile builds RAW
    edges from emission order); absorber ops (aabs/dabs) pad pipeline
    distance / pull cross-engine sems into the right clocks so the
    walrus single-wait legalizer can prune every instruction to one
    semaphore wait.
  - drains: whole [128,1000] bf16 psum->SBUF per (blk,p), engine
    chosen per psum slot (slot=i%6: DVE {0,1,4}, ACT {2,3,5}) so slot
    WAR stays on one engine.  First two gathers run in 2 virgin psum
    banks (pool A) so they need no lin-bank reuse observer.
  - one contiguous 256KB output DMA per (blk,p): even on the sync
    ring (reusing input lanes whose completion is implied through the
    compute chain), odd on the Pool SWDGE ring behind a Pool observer
    that carries the data wait.
  - _install_drain_split post-processes the serialized BIR with a
    vector-clock pass that drops transitively-implied semaphore waits
    and splits the tail Drains, enforcing the single-wait encoding.
"""

import numpy as np

B, P, Q, C, F = 256, 64, 128, 1000, 2048
NCORES = 8
PL = P // NCORES          # partitionings per core
KT2 = 8                   # K tiles of 256 (128 partitions x DoubleRow 2)
XC = B                    # x columns in the xw stream
WC = PL * Q               # W columns in the xw stream
NBLK = B // 128           # b blocks
WSCALE = 32.0
N_WARM = 3                # PE clock pre-ramp matmuls


def _build_nc():
    import concourse.bass as bass
    import concourse.tile as tile
    from concourse import mybir
    from contextlib import ExitStack

    DT = mybir.dt.float32
    BF = mybir.dt.bfloat16
    F8 = mybir.dt.float8e4
    ACT = mybir.ActivationFunctionType

    nc = bass.Bass()
    xw_d = nc.declare_dram_parameter("xw", [KT2, 128, 2, XC + WC], F8,
                                     isOutput=False)
    bo_d = nc.declare_dram_parameter("bo", [1, WC + 128], BF, isOutput=False)
    id_d = nc.declare_dram_parameter("ident", [128, 128], BF, isOutput=False)
    oh_d = nc.declare_dram_parameter("oh", [PL, 128, C], BF,
                                     isOutput=False)
    out_d = nc.declare_dram_parameter("out", [PL, NBLK * 128, C], BF,
                                      isOutput=True)

    with ExitStack() as ctx:
        tc = ctx.enter_context(tile.TileContext(nc))
        singles = ctx.enter_context(tc.tile_pool(name="singles", bufs=1))
        ps_outA = ctx.enter_context(
            tc.tile_pool(name="ps_outA", bufs=2, space=bass.MemorySpace.PSUM))
        ps_t = ctx.enter_context(
            tc.tile_pool(name="ps_t", bufs=1, space=bass.MemorySpace.PSUM))
        lin_ctx = ExitStack()
        ps_lin = lin_ctx.enter_context(
            tc.tile_pool(name="ps_lin", bufs=1, space=bass.MemorySpace.PSUM))

        def fresh(shape, dtype, tag):
            return singles.tile(shape, dtype, tag=tag, name=tag)

        # ---- input DMAs (sync HWDGE ring, priority order) -----------
        # bo first (gates the openers), then the xw k-tiles, ident, and
        # the one-hot chunks (bf16 in DRAM) which fill the otherwise
        # idle DMA window between the input and output streams.
        bo_sb = fresh([1, WC + 128], BF, "bo")
        nc.sync.dma_start(out=bo_sb[:], in_=bo_d[:])
        xwk = []
        for k in range(KT2):
            t = fresh([128, 2, XC + WC], F8, f"xwk{k}")
            nc.sync.dma_start(out=t[:], in_=xw_d[k])
            xwk.append(t)
        # ident is only needed by the transposes (after the mains), so
        # it rides behind the k-tiles
        id_sb = fresh([128, 128], BF, "ident")
        nc.sync.dma_start(out=id_sb[:], in_=id_d[:])
        oh_bf = fresh([128, PL * C], BF, "ohbf")
        for p in range(PL):
            nc.sync.dma_start(out=oh_bf[:, p * C:(p + 1) * C], in_=oh_d[p])
        # raw (untracked, uninitialized) SBUF tensor for warmups:
        # garbage data, zero dependencies, so the PE clock ramp starts
        # the moment the engine is free
        warm_t = nc.alloc_sbuf_tensor("warmjunk", [128, 256], DT)

        pst = {}
        for blk in range(NBLK):
            pst[blk] = ps_t.tile([128, PL, 128], BF, name=f"pst{blk}")

        lin = {}
        for blk in range(NBLK):
            for h in range(2):
                lin[(blk, h)] = ps_lin.tile([128, 4, 128], DT,
                                            name=f'lin{blk}_{h}')

        # ---- PE: clock warmups, bias openers ------------------------
        # warmups: fp32 garbage matmuls (4 cycles/row, long-running, no
        # deps) into lin00, which its opener then overwrites
        for w in range(N_WARM):
            nc.tensor.matmul(
                lin[(0, 0)][0:1, 0:2, :],
                warm_t.ap()[:, 0:1],
                warm_t.ap()[:, 0:256],
                start=True, stop=True)
        for blk in range(NBLK):
            for h in range(2):
                nc.tensor.matmul(
                    lin[(blk, h)][:, :, :],
                    bo_sb[0:1, WC:WC + 128],
                    bo_sb[0:1, h * 512:(h + 1) * 512],
                    start=True, stop=False)

        # ---- PE mains: k-major, DoubleRow fp8 -----------------------
        for k in range(KT2):
            for blk in range(NBLK):
                for h in range(2):
                    nc.tensor.matmul(
                        lin[(blk, h)][:, :, :],
                        xwk[k][:, :, blk * 128:(blk + 1) * 128],
                        xwk[k][:, :, XC + h * 512:XC + (h + 1) * 512],
                        start=False, stop=(k == KT2 - 1),
                        perf_mode=mybir.MatmulPerfMode.DoubleRow)

        # dummy transpose consumes ident's DMA sem before the real
        # transposes (keeps them at one wait each)
        nc.tensor.transpose(pst[0][:, 0, :], id_sb[:], id_sb[:])


        # ---- softmax: linsc+copies early on DVE; exp/ln/neg on ACT --
        linsc, exps, sumexp, lse, neg_lse = {}, {}, {}, {}, {}
        for blk in range(NBLK):
            sumexp[blk] = fresh([128, PL], DT, f"sumexp{blk}")
            lse[blk] = fresh([128, PL], DT, f"lse{blk}")
            neg_lse[blk] = fresh([128, PL], DT, f"neglse{blk}")
        for blk in range(NBLK):
            for h in range(2):
                linsc[(blk, h)] = fresh([128, 4, 128], BF,
                                        f"linsc{blk}_{h}")
                exps[(blk, h)] = fresh([128, 4, 128], BF, f"exp{blk}_{h}")

        def mk_linsc(blk, h):
            nc.vector.tensor_scalar_mul(linsc[(blk, h)][:, :, :],
                                        lin[(blk, h)][:, :, :],
                                        1.0 / WSCALE)

        def mk_exp(blk, h):
            nc.scalar.activation(out=exps[(blk, h)][:, :, :],
                                 in_=lin[(blk, h)][:, :, :],
                                 func=ACT.Exp, scale=1.0 / WSCALE)

        def mk_red(blk, h):
            nc.vector.tensor_reduce(
                out=sumexp[blk][:, h * 4:(h + 1) * 4],
                in_=exps[(blk, h)][:, :, :],
                axis=mybir.AxisListType.X, op=mybir.AluOpType.add)

        def mk_lse(blk):
            nc.scalar.activation(out=lse[blk][:, :], in_=sumexp[blk][:, :],
                                 func=ACT.Ln)
            nc.scalar.activation(out=neg_lse[blk][:, :], in_=lse[blk][:, :],
                                 func=ACT.Identity, scale=-1.0)

        logpT = {}

        def mk_T(blk, h):
            for p in range(h * 4, h * 4 + 4):
                nc.tensor.transpose(pst[blk][:, p, :],
                                    linsc[(blk, h)][:, p % 4, :],
                                    id_sb[:])
            t = fresh([128, 4, 128], BF, f"logpT{blk}_{h}")
            logpT[(blk, h)] = t
            nc.vector.tensor_copy(t[:, :, :],
                                  pst[blk][:, h * 4:(h + 1) * 4, :])

        # Emission follows dataflow (tile builds RAW edges from
        # emission order).  Gathers are split into matmul and drain
        # emission so the first gathers can be emitted before the lse
        # chain, with their drains after it.
        aabs = fresh([1, 24], DT, "aabs")
        dabs = fresh([128, 24], DT, "dabs")
        obs_junk = fresh([1, 16], DT, "obs_junk")

        # gather bookkeeping ------------------------------------------
        # slot = i % 6; slots 0,1 live in ps_outA (virgin banks), 2-5
        # in ps_outB (freed lin banks).  Drain engine per slot: DVE
        # {0,1,4}, ACT {2,3,5}.
        DVE_SLOTS = {0, 1, 4}
        SEQ = [(0, 0), (0, 1), (0, 2), (0, 3), (1, 0), (1, 1),
               (1, 2), (1, 3), (0, 4), (0, 5), (0, 6), (0, 7),
               (1, 4), (1, 5), (1, 6), (1, 7)]
        po_of, og_dma = {}, {}
        seen_oh = set()
        ps_outB = None

        def gather_mm(i):
            blk, p = SEQ[i]
            if p not in seen_oh:
                # absorb this oh chunk's DMA sem
                seen_oh.add(p)
                nc.tensor.ldweights(oh_bf[:, p * C:p * C + 1])
            pool = ps_outA if i % 6 < 2 else ps_outB
            po = pool.tile([128, C], BF, name='po')
            po_of[i] = po
            for c0, cw in ((0, 512), (512, 488)):
                nc.tensor.matmul(
                    po[:, c0:c0 + cw], logpT[(blk, p // 4)][:, p % 4, :],
                    oh_bf[:, p * C + c0:p * C + c0 + cw],
                    is_transpose=True, start=True, stop=True)

        def gather_drain(i):
            blk, p = SEQ[i]
            po = po_of[i]
            og = fresh([128, C], BF, f"og{blk}_{p}")
            if i % 6 in DVE_SLOTS:
                nc.vector.tensor_scalar(
                    out=og[:, :], in0=po[:, :],
                    scalar1=lse[blk][:, p:p + 1], scalar2=None,
                    op0=mybir.AluOpType.subtract)
            else:
                nc.scalar.activation(
                    out=og[:, :], in_=po[:, :],
                    func=ACT.Identity, scale=1.0,
                    bias=neg_lse[blk][:, p:p + 1])
            bsl = slice(blk * 128, (blk + 1) * 128)
            if i % 2 == 0:
                # sync ring: <=8 outs, each reuses an input lane whose
                # completion is implied through the compute chain
                dma = nc.sync.dma_start(out=out_d[p, bsl, :], in_=og[:])
            else:
                # pool ring: observer carries the data wait, freeing
                # the dma's single wait slot for its lane wait
                obs = nc.gpsimd.tensor_copy(obs_junk[0:1, i:i + 1],
                                            og[0:1, C - 1:C])
                dma = nc.gpsimd.dma_start(out=out_d[p, bsl, :], in_=og[:])
                tile.add_dep_helper(dma.ins, obs.ins, sync=False,
                                    reason="dma after pool obs")
            og_dma[i] = (og, dma)

        # ---- interleaved softmax + transpose + gather emission ------
        mk_linsc(0, 0)
        mk_exp(0, 0)
        mk_T(0, 0)
        gather_mm(0)
        gather_mm(1)
        mk_exp(0, 1)
        mk_exp(1, 0)
        mk_exp(1, 1)
        mk_red(0, 0)
        mk_red(0, 1)
        mk_lse(0)
        nc.vector.tensor_copy(dabs[:, 0:1], lse[0][:, 0:1])
        nc.scalar.activation(out=aabs[0:1, 0:1],
                             in_=neg_lse[0][0:1, 0:1],
                             func=ACT.Copy, bias=0.0, scale=1.0)
        gather_drain(0)
        gather_drain(1)
        mk_linsc(1, 0)
        mk_T(1, 0)
        mk_red(1, 0)
        mk_linsc(0, 1)
        mk_T(0, 1)
        mk_red(1, 1)
        mk_linsc(1, 1)
        mk_T(1, 1)

        # lin banks free -> 4 more one-bank gather slots
        lin_ctx.close()
        ps_outB = ctx.enter_context(
            tc.tile_pool(name="ps_outB", bufs=4, space=bass.MemorySpace.PSUM))

        # PE observer: absorb ACT's lin-bank reads (exps) so gather
        # bank-reuse WARs are covered by engine order
        nc.tensor.ldweights(exps[(1, 1)][:, 3, 0:1])

        # blk0 gathers i2,i3 emit before the lse(1) chain so ACT's
        # first drains aren't queued behind ln(1) in ACT order
        for i in (2, 3):
            gather_mm(i)
            gather_drain(i)
        mk_lse(1)
        nc.vector.tensor_copy(dabs[:, 1:2], lse[1][:, 0:1])
        nc.scalar.activation(out=aabs[0:1, 1:2],
                             in_=neg_lse[1][0:1, 0:1],
                             func=ACT.Copy, bias=0.0, scale=1.0)
        for i in range(4, 16):
            gather_mm(i)
            gather_drain(i)

    _install_drain_split(nc)
    return nc


def _install_drain_split(nc, chunk=1):
    """Legalize sync for this walrus build (at most ONE sync wait per
    instruction):

    1. Vector-clock pass: compute, for every instruction, the set of
       instructions provably COMPLETED before it dispatches — via its
       own sem waits (a wait S>=v proves every update contributing to
       values 1..v completed, and transitively everything those
       instructions' dispatch-clocks contain) plus same-engine dispatch
       order (an engine dispatches in program order, so anything done
       before a predecessor's dispatch is done before ours).  Any
       emitted wait already implied by the rest is dropped.
    2. Remaining multi-wait Drains are split into single-wait chains.
    """
    import copy
    import json

    orig = nc.to_json_bytes

    def patched():
        m = json.loads(orig())
        for fn in m["functions"]:
            insts = []
            for bb in fn["blocks"]:
                insts.extend(bb["instructions"])
            n = len(insts)
            # sem name -> list of (cum_value, idx) in completion order
            updates = {}
            cum = {}
            for i, inst in enumerate(insts):
                si = inst.get("sync_info") or {}
                for u in (si.get("on_update") or []):
                    s = u["ant_name"]
                    cum[s] = cum.get(s, 0) + u.get("update_value", 1)
                    updates.setdefault(s, []).append((cum[s], i))

            def targets(s, v):
                """instruction idxs whose updates are needed for sem s
                to reach v"""
                return [i for (c, i) in updates.get(s, []) if c <= v]

            eng_pred = {}
            last = {}
            for i, inst in enumerate(insts):
                e = inst.get("engine", "?")
                eng_pred[i] = last.get(e)
                last[e] = i

            done = [set() for _ in range(n)]
            for _ in range(64):
                changed = False
                for i, inst in enumerate(insts):
                    d = set()
                    if eng_pred[i] is not None:
                        p = eng_pred[i]
                        d |= done[p]
                    si = inst.get("sync_info") or {}
                    for w in (si.get("on_wait") or []):
                        for j in targets(w["ant_name"], w["wait_value"]):
                            d.add(j)
                            d |= done[j]
                    if d != done[i]:
                        done[i] = d
                        changed = True
                if not changed:
                    break

            # drop implied waits on multi-wait instructions
            for i, inst in enumerate(insts):
                si = inst.get("sync_info") or {}
                waits = si.get("on_wait") or []
                if len(waits) <= 1:
                    continue
                base = set()
                if eng_pred[i] is not None:
                    base |= done[eng_pred[i]]
                keep = list(waits)
                for w in list(keep):
                    others = set(base)
                    for w2 in keep:
                        if w2 is w:
                            continue
                        for j in targets(w2["ant_name"], w2["wait_value"]):
                            others.add(j)
                            others |= done[j]
                    if all(j in others
                           for j in targets(w["ant_name"], w["wait_value"])):
                        keep.remove(w)
                        if len(keep) <= 1:
                            break
                si["on_wait"] = keep

        # split any remaining multi-wait Drains
        for fn in m["functions"]:
            for bb in fn["blocks"]:
                out = []
                for inst in bb["instructions"]:
                    si = inst.get("sync_info")
                    if (si and si.get("on_wait")
                            and len(si["on_wait"]) > chunk):
                        if inst.get("opcode") != "Drain":
                            raise RuntimeError(
                                f"multi-wait survives legalization: "
                                f"{inst.get('opcode')} {inst.get('name')} "
                                f"{si['on_wait']}")
                        waits = si["on_wait"]
                        head, keep = waits[:-chunk], waits[-chunk:]
                        for j in range(0, len(head), chunk):
                            clone = copy.deepcopy(inst)
                            clone["name"] = f"{inst['name']}-ds{j}"
                            clone["sync_info"] = {
                                "on_wait": head[j:j + chunk],
                                "on_update": [],
                            }
                            out.append(clone)
                        si["on_wait"] = keep
                    out.append(inst)
                bb["instructions"] = out
        return json.dumps(m).encode()

    nc.to_json_bytes = patched


def _host_inputs(x, W, b, part_idx):
    import ml_dtypes
    f8 = ml_dtypes.float8_e4m3
    bf = ml_dtypes.bfloat16

    # x: [B, F] -> [KT2, 128, 2, B] with f = 256*k2 + 128*j + r
    xT = np.ascontiguousarray(
        x.T.reshape(KT2, 2, 128, B).transpose(0, 2, 1, 3)).astype(f8)
    ident = np.eye(128, dtype=np.float32).astype(bf)
    qarange = np.arange(Q, dtype=np.int64)

    in_maps = []
    for i in range(NCORES):
        sl = slice(i * PL, (i + 1) * PL)
        Wt = (W[sl] * WSCALE).transpose(2, 0, 1).reshape(
            KT2, 2, 128, PL * Q).transpose(0, 2, 1, 3)
        xw = np.empty((KT2, 128, 2, XC + WC), dtype=f8)
        xw[:, :, :, :XC] = xT
        xw[:, :, :, XC:] = Wt.astype(f8)
        bo = np.empty((1, WC + 128), dtype=bf)
        bo[0, :WC] = (b[sl].reshape(-1) * WSCALE).astype(bf)
        bo[0, WC:] = 1.0
        # oh[p, q, c] = 1 iff part_idx[p_global, c] == q
        oh = (qarange[None, :, None] == part_idx[sl][:, None, :]
              ).astype(bf)
        in_maps.append({"xw": xw, "bo": bo, "ident": ident, "oh": oh})
    return in_maps


def kernel(x, W, b, part_idx, _trace=False):
    from concourse.bass_utils import run_bass_kernel_spmd

    x = np.asarray(x, dtype=np.float32)
    W = np.asarray(W, dtype=np.float32)
    b = np.asarray(b, dtype=np.float32)
    part_idx = np.asarray(part_idx)

    nc = _build_nc()
    in_maps = _host_inputs(x, W, b, part_idx)
    res = run_bass_kernel_spmd(nc, in_maps, list(range(NCORES)),
                               trace=_trace)
    out = np.empty((B, P, C), dtype=np.float32)
    for i, r in enumerate(res.results):
        # r["out"]: [PL, 256, 1000] bf16, b-major rows
        o = np.asarray(r["out"], dtype=np.float32)
        out[:, i * PL:(i + 1) * PL, :] = o.transpose(1, 0, 2)
    if _trace:
        return out, res
    return out
